# revision 23
# baseline (speedup 1.0000x reference)
"""Trainium2 Bass kernel for nn_AverageAttention: cumulative-average attention
with a sigmoid gating Linear(2D->2D).

Strategy: data-parallel over batch (B=8 = one batch element per NeuronCore).
All on-chip work happens in transposed space ([feature, token]); the gating
matmul runs in fp8e4 DoubleRow mode (2 fp8 MACs/cell/cycle, 256-deep
contraction per matmul) which halves the PE-array time vs a bf16 version:
  - host pre-scales W by 1024 (keeps fp8e4 values in the normal range) and
    packs it into per-unit tiles [p, kp, pair, 256] so each DoubleRow
    LDWEIGHTS reads one contiguous-stride 3D AP; the sigmoid activation's
    scale=1/1024 undoes the scaling for free
  - x is cast to fp8 host-side; avg is produced by the on-chip scan (bf16
    out, f32 carry) and cast to fp8 (ScalarE for slice-pair 0+1, VectorE
    for 2+3 where ScalarE is busy with sigmoids); fp8 quantization of the
    matmul operands lands at ~1.4e-2 relative on the gating output (vs the
    2e-2 gate), verified in a host-side simulation
  - cumavg via the affine recurrence avg_t = coef_t*avg_{t-1} + x_t/(t+1),
    one fused tensor_tensor_scan per 512-col chunk on VectorE (the scan is
    DVE-only and ~1.2us/chunk serial, so the 32-chunk chain for slice-pair
    0+1 bounds the startup; slice-pair 2+3 scans are spread over the unit
    windows of phase B1)
  - engine/queue discipline (every DMA issue costs ~0.6us on its issuing
    engine, so transfers are pair-merged): sync = W + x8 + xbf + bias
    (pure loads, never wait), gpsimd = xd/coef loads then avg-out writes
    (all slice-0+1 loads issued before any scan-gated write), scalar/ACT =
    fp8 casts + sigmoids + gating-out writes, nothing queued ahead of them
  - matmul schedule: two phases over t (slices {0,1} then {2,3}); per unit
    (128 gate features x {ig,fg}) each weight k-pair feeds 2 matmuls so the
    DoubleRow LDWEIGHTS hides behind 2 matmuls; PSUM = 4 banks per unit,
    2 units in flight; units 0-1 issue their x-half matmuls as a scan-free
    runway, then consume their avg halves slice-0-first so the PE streams
    while the slice-1 scan chain drains; phase B2 walks units in reverse
    so the last 4 W tiles are reused from the 5-deep pool and fresh loads
    always alias already-consumed buffers
  - epilogue: sigmoid+bias+descale on ScalarE reading PSUM, combine on
    VectorE (streamed bf16 x, resident fp8 avg), both t-slices of a unit
    written back in one transposed bf16 DMA and un-transposed on host.
"""
import sys

if "/opt/trn_rl_repo" not in sys.path:
    sys.path.insert(0, "/opt/trn_rl_repo")

import numpy as np
import ml_dtypes

B, T, D = 8, 2048, 2048
O = 2 * D          # gate output features (4096)
P = 128            # partitions
KT = D // P        # 16 feature tiles per half of G
KP = 16            # DoubleRow k-pairs per gate (2D/256)
DT = D // P        # 16 units (gate-feature tiles, ig+fg pair each)
TS = 512           # t-slice (matmul moving free dim / scan chunk)
NS = T // TS       # 4 t-slices
HT = T // 2        # column split between phases B1/B2
WS = 1024.0        # host-side W scale (undone in the sigmoid activation)

_compiled = None


def _build():
    import concourse.mybir as mybir
    import concourse.tile as tile
    from concourse import bacc

    f32 = mybir.dt.float32
    bf16 = mybir.dt.bfloat16
    f8 = mybir.dt.float8e4
    SIG = mybir.ActivationFunctionType.Sigmoid
    CPY = mybir.ActivationFunctionType.Copy
    DR = mybir.MatmulPerfMode.DoubleRow

    nc = bacc.Bacc(trn_type="TRN2", target_bir_lowering=False, debug=False,
                   num_devices=B)

    x8_d = nc.declare_dram_parameter("x8T", [D, T], f8, isOutput=False)
    xb_d = nc.declare_dram_parameter("xbT", [D, T], bf16, isOutput=False)
    xd_d = nc.declare_dram_parameter("xdT", [D, T], bf16, isOutput=False)
    wP_d = nc.declare_dram_parameter("wP", [DT, P, KP, 2, 256], f8,
                                     isOutput=False)
    bias_d = nc.declare_dram_parameter("bias", [O], f32, isOutput=False)
    coef_d = nc.declare_dram_parameter("coef_t", [1, T], f32, isOutput=False)
    avgT_d = nc.declare_dram_parameter("avgT", [D, T], bf16, isOutput=True)
    outT_d = nc.declare_dram_parameter("outT", [D, T], bf16, isOutput=True)

    with tile.TileContext(nc) as tc:
        with tc.tile_pool(name="consts", bufs=1) as consts, \
             tc.tile_pool(name="resid", bufs=1) as resid, \
             tc.tile_pool(name="xdp", bufs=16) as xdp, \
             tc.tile_pool(name="avp", bufs=6) as avp, \
             tc.tile_pool(name="xbp", bufs=6) as xbp, \
             tc.tile_pool(name="wpool", bufs=5) as wpool, \
             tc.tile_pool(name="sigp", bufs=10) as sigp, \
             tc.tile_pool(name="outp", bufs=6) as outp, \
             tc.tile_pool(name="psum", bufs=8, space="PSUM") as pp:

            coef_sb = consts.tile([P, T], f32)
            bias_sb = consts.tile([P, 2 * DT], f32)
            carry = consts.tile([P, KT], f32)

            x8 = resid.tile([P, KT, T], f8)
            a8 = resid.tile([P, KT, T], f8)

            def x8_pair(jp, c0, c1, eng=None):
                src = x8_d[2 * jp * P:(2 * jp + 2) * P, c0:c1].rearrange(
                    "(j p) c -> p j c", p=P)
                (eng or nc.sync).dma_start(out=x8[:, 2 * jp:2 * jp + 2,
                                                  c0:c1], in_=src)

            def load_w(u, split=2):
                w_u = wpool.tile([P, KP, 2, 256], f8, tag="w")
                src = wP_d[u]
                step = KP // split
                for c in range(split):
                    ks = slice(c * step, (c + 1) * step)
                    nc.sync.dma_start(out=w_u[:, ks, :, :],
                                      in_=src[:, ks, :, :])
                return w_u

            # --- sync queue head: W unit 0 interleaved with the x8 pairs
            # --- the runway consumes in the same k order
            # x8 pairs 0-3 ride the sync queue 1:1 with w0 chunks; pairs
            # 4-7 go via gpsimd (between the xd loads) so the sync queue's
            # issue rate doesn't pace the matmul runway.
            w = {}
            w[0] = wpool.tile([P, KP, 2, 256], f8, tag="w", name="w_0")
            for c in range(8):
                ks = slice(2 * c, 2 * c + 2)
                nc.sync.dma_start(out=w[0][:, ks, :, :],
                                  in_=wP_d[0][:, ks, :, :])
                if c < 4:
                    x8_pair(c, 0, HT)
            nc.sync.dma_start(
                out=bias_sb, in_=bias_d.rearrange("(c p) -> p c", p=P))
            w[1] = load_w(1, split=4)
            w[2] = load_w(2)
            w[3] = load_w(3)
            w[4] = load_w(4)

            def load_xb(u, s0):
                xc = xbp.tile([P, 2 * TS], bf16, tag="xb")
                nc.sync.dma_start(
                    out=xc, in_=xb_d[u * P:(u + 1) * P,
                                     s0 * TS:(s0 + 2) * TS])
                return xc

            def scan_load(s):
                sl = slice(s * TS, (s + 1) * TS)
                nc.gpsimd.dma_start(
                    out=coef_sb[:, sl],
                    in_=coef_d[:, sl].to_broadcast((P, TS)))
                xds = []
                for jp in range(KT // 2):
                    xd = xdp.tile([P, 2, TS], bf16, tag="xd")
                    src = xd_d[2 * jp * P:(2 * jp + 2) * P, sl].rearrange(
                        "(j p) c -> p j c", p=P)
                    nc.gpsimd.dma_start(out=xd, in_=src)
                    xds.append(xd)
                return xds

            def scan_one(eng, s, j, xds, cast_eng):
                sl = slice(s * TS, (s + 1) * TS)
                av = avp.tile([P, TS], bf16, tag="av", name=f"av_{s}_{j}")
                eng.tensor_tensor_scan(
                    out=av, data0=coef_sb[:, sl], data1=xds[j // 2][:, j % 2, :],
                    initial=(0.0 if s == 0 else carry[:, j:j + 1]),
                    op0=mybir.AluOpType.mult, op1=mybir.AluOpType.add)
                if s < NS - 1:
                    eng.tensor_copy(carry[:, j:j + 1], av[:, TS - 1:TS])
                if cast_eng is nc.scalar:
                    nc.scalar.activation(a8[:, j, sl], av, CPY)
                else:
                    cast_eng.tensor_copy(a8[:, j, sl], av)
                return av

            def scan_compute_early(s, xds):
                """Slice-pair 0+1: scans on DVE (the serial recurrence is
                DVE-only and ~1.2us/512 regardless of dtype), casts on the
                otherwise-idle ScalarE, avg-out writes on gpsimd."""
                sl = slice(s * TS, (s + 1) * TS)
                for j in range(KT):
                    av = scan_one(nc.vector, s, j, xds, nc.scalar)
                    nc.gpsimd.dma_start(out=avgT_d[j * P:(j + 1) * P, sl],
                                        in_=av)

            def scan_compute_late(s, xds, jlo, jhi):
                """Slice-pair 2+3: scans on DVE, casts on ScalarE, avg-out
                writes on gpsimd; emitted in half-sets so neither DVE's nor
                ScalarE's backlog per matmul-unit window exceeds the unit
                time (which would delay epilogues and stall PSUM reuse)."""
                sl = slice(s * TS, (s + 1) * TS)
                for j in range(jlo, jhi):
                    av = scan_one(nc.vector, s, j, xds, nc.scalar)
                    nc.gpsimd.dma_start(out=avgT_d[j * P:(j + 1) * P, sl],
                                        in_=av)

            def mm(ps, w_u, g, kp, s, start, stop):
                lhsT = w_u[:, kp, :, g * 128:(g + 1) * 128]
                if kp < KP // 2:
                    rhs = x8[:, 2 * kp:2 * kp + 2, s * TS:(s + 1) * TS]
                else:
                    q = kp - KP // 2
                    rhs = a8[:, 2 * q:2 * q + 2, s * TS:(s + 1) * TS]
                nc.tensor.matmul(ps, lhsT=lhsT, rhs=rhs, start=start,
                                 stop=stop, perf_mode=DR)

            def alloc_ps():
                # [gate][slice-in-pair]
                return [[pp.tile([P, TS], f32, tag="ps", name=f"ps_{g}_{si}")
                         for si in range(2)] for g in range(2)]

            def mm_range(ps, w_u, ss, kplo, kphi):
                for g in range(2):
                    for kp in range(kplo, kphi):
                        for si, s in enumerate(ss):
                            mm(ps[g][si], w_u, g, kp, s,
                               start=(kp == 0), stop=(kp == KP - 1))

            def epilogue(ps, u, ss, xbc):
                out_u = outp.tile([P, 2 * TS], bf16, tag="out")
                for si, s in enumerate(ss):
                    sl = slice(s * TS, (s + 1) * TS)
                    osl = slice(si * TS, (si + 1) * TS)
                    sig_i = sigp.tile([P, TS], f32, tag="sig")
                    nc.scalar.activation(sig_i, ps[0][si], SIG,
                                         bias=bias_sb[:, u:u + 1],
                                         scale=1.0 / WS)
                    sig_f = sigp.tile([P, TS], f32, tag="sig")
                    nc.scalar.activation(sig_f, ps[1][si], SIG,
                                         bias=bias_sb[:, DT + u:DT + u + 1],
                                         scale=1.0 / WS)
                    nc.vector.tensor_mul(out_u[:, osl], sig_i, xbc[:, osl])
                    nc.vector.tensor_mul(sig_f, sig_f, a8[:, u, sl])
                    nc.vector.tensor_add(out_u[:, osl], out_u[:, osl], sig_f)
                nc.scalar.dma_start(
                    out=outT_d[u * P:(u + 1) * P,
                               ss[0] * TS:(ss[0] + 2) * TS],
                    in_=out_u)

            # ---- scan slice-pair (0,1): loads fully ahead of compute ----
            xds0 = scan_load(0)
            for jp in range(4, KT // 2):
                x8_pair(jp, 0, HT, eng=nc.gpsimd)
            xds1 = scan_load(1)
            scan_compute_early(0, xds0)
            scan_compute_early(1, xds1)

            # ---- phase B1: t-slices (0, 1) ------------------------------
            ss = (0, 1)
            ps01 = {0: alloc_ps(), 1: alloc_ps()}
            xb01 = {0: load_xb(0, 0), 1: load_xb(1, 0)}
            # runway: x-half matmuls of units 0-1 run while scans finish;
            # their avg halves then consume slice 0 first (its casts land
            # ~19us before slice 1's) so the PE keeps streaming while the
            # slice-1 scan chain drains
            for u in (0, 1):
                mm_range(ps01[u], w[u], ss, 0, KP // 2)
            for si in range(2):
                for u in (0, 1):
                    for g in range(2):
                        for kp in range(KP // 2, KP):
                            mm(ps01[u][g][si], w[u], g, kp, ss[si],
                               start=False, stop=(kp == KP - 1))
            for u in (0, 1):
                epilogue(ps01[u], u, ss, xb01[u])
                if u == 0:
                    xds2 = scan_load(2)
            for u in range(2, DT):
                ps_u = alloc_ps()
                xbc = load_xb(u, 0)
                mm_range(ps_u, w[u], ss, 0, KP)
                if u + 3 <= DT - 1:
                    w[u + 3] = load_w(u + 3)
                if u == 2:
                    scan_compute_late(2, xds2, 0, 6)
                if u == 3:
                    scan_compute_late(2, xds2, 6, 11)
                    xds3 = scan_load(3)
                if u == 4:
                    scan_compute_late(2, xds2, 11, KT)
                if u == 5:
                    scan_compute_late(3, xds3, 0, 6)
                if u == 6:
                    scan_compute_late(3, xds3, 6, 11)
                if u == 7:
                    scan_compute_late(3, xds3, 11, KT)
                if u == 6:
                    for jp in range(KT // 2):
                        x8_pair(jp, HT, T)
                epilogue(ps_u, u, ss, xbc)

            # ---- phase B2: t-slices (2, 3), reverse unit order ----------
            # The last W tiles from B1 (u=12..15) are still resident in the
            # 5-deep weight pool, so the reversed order skips their reloads.
            # Fresh loads follow the pool's buffer rotation: load_w(u-2)
            # emitted after unit u's matmuls always aliases a W tile whose
            # readers are already emitted (w11 re-aliases its own old
            # buffer), keeping ~1.5 unit-times of DMA lead throughout.
            ss = (2, 3)
            w[11] = load_w(11)
            for u in range(DT - 1, -1, -1):
                ps_u = alloc_ps()
                xbc = load_xb(u, 2)
                mm_range(ps_u, w[u], ss, 0, KP)
                if 2 <= u <= 12:
                    w[u - 2] = load_w(u - 2)
                epilogue(ps_u, u, ss, xbc)

    nc.compile()
    return nc


def _get_compiled():
    global _compiled
    if _compiled is None:
        _compiled = _build()
    return _compiled


def _run(inputs, trace=False, **spmd_kwargs):
    from concourse.bass_utils import run_bass_kernel_spmd

    nc = _get_compiled()
    layer_in = np.asarray(inputs["layer_in"], dtype=np.float32)
    W_gate = np.asarray(inputs["W_gate"], dtype=np.float32)
    b_gate = np.asarray(inputs["b_gate"], dtype=np.float32)

    f8 = ml_dtypes.float8_e4m3
    bf16 = ml_dtypes.bfloat16

    # wP[u, p, kp, j, m]: m<128 -> ig rows (o = u*128+m), m>=128 -> fg rows
    # (o = D+u*128+m-128); contraction k = kp*256 + j*128 + p; scaled by WS.
    Wig = W_gate[:D].reshape(DT, P, O)
    Wfg = W_gate[D:].reshape(DT, P, O)
    Wcat = np.concatenate([Wig, Wfg], axis=1)            # [u, 256(m), k]
    wP = np.ascontiguousarray(
        Wcat.reshape(DT, 256, KP, 2, P).transpose(0, 4, 2, 3, 1)) * WS
    wP = np.clip(wP, -240.0, 240.0).astype(f8)

    tt = np.arange(T, dtype=np.float32)
    coef = (tt / (tt + 1.0)).reshape(1, T)
    inv = (1.0 / (tt + 1.0)).reshape(1, T)

    in_maps = []
    for b in range(B):
        xTb = np.ascontiguousarray(layer_in[b].T)
        in_maps.append({
            "x8T": np.clip(xTb, -240.0, 240.0).astype(f8),
            "xbT": xTb.astype(bf16),
            "xdT": (xTb * inv).astype(bf16),
            "wP": wP,
            "bias": b_gate,
            "coef_t": coef,
        })

    res = run_bass_kernel_spmd(nc, in_maps, core_ids=list(range(B)),
                               trace=trace, **spmd_kwargs)
    gating = np.empty((B, T, D), dtype=np.float32)
    avg = np.empty((B, T, D), dtype=np.float32)
    for b in range(B):
        gating[b] = res.results[b]["outT"].astype(np.float32).T
        avg[b] = res.results[b]["avgT"].astype(np.float32).T
    return (gating, avg), res


def kernel(**inputs):
    (gating, avg), _ = _run(inputs, trace=False)
    return gating, avg
